# revision 10
# baseline (speedup 1.0000x reference)
"""Bahdanau-style attention with coverage (sparse_attention) on 8 TRN2 cores.

Math per example (B=32, TX=2048, DH=512, U2=1024):
    enc   = a @ Wa                      [TX, U2]
    dec   = h @ Wh_w + Wh_b             [U2]
    feats = enc + dec + coverage[:,None]*Wc
    e     = tanh(feats) @ v             [TX]
    alpha = masked_softmax(e)           [TX]   (softmax, mask, renormalize)
    ctx   = alpha @ a                   [U2]

Sharding: data-parallel over batch, 4 examples per core, no collectives.

Device schedule (per example, t-tile j covers tokens j*128..j*128+127):
    main matmul computes enc tiles [t=128p, u=1024f] in PSUM:
        out = lhsT.T @ rhs, lhsT = aT tile [k,t] (host-transposed a, bf16),
        rhs = Wa [k, u] (bf16).  coverage/dec terms are folded in as an
        extra K=2 accumulation: lhsT=[cov_row; ones], rhs=[Wc; dec].
    tanh on ACT (PSUM -> SBUF bf16); e-projection = DVE tensor_mul by a
    v broadcast + reduce_sum over the free dim -> e column [128,1].
    softmax skips max-subtraction (|e| <= ||v||_1 ~ 26, safe in fp32);
    S = sum_t exp(e)*mask is broadcast to all 128 partitions with a
    ones[128,128] fp32 matmul, then alpha = pm * (1/S) per partition.
    context matmul: lhsT = alpha column [128,1] bf16, rhs = a natural
    [t=128p, u] (second host input), accumulated over 16 t-chunks.
    The previous example's softmax/context tail is emitted inside the
    next example's main loop so PE never stalls on the serial tail.
"""

import numpy as np
import ml_dtypes
from contextlib import ExitStack

import concourse.bass as bass
import concourse.bacc as bacc
import concourse.tile as tile
from concourse import mybir
from concourse.bass_utils import run_bass_kernel_spmd

BF16 = mybir.dt.bfloat16
F32 = mybir.dt.float32
NP_BF16 = ml_dtypes.bfloat16

B, TX, DH, U2 = 32, 2048, 512, 1024
NCORES = 8
EX = B // NCORES          # 4 examples per core
NT = TX // 128            # 16 t-tiles
NK = U2 // 128            # 8 k-chunks (contraction for enc)
NKH = DH // 128           # 4 k-chunks (contraction for dec)

_TANH = mybir.ActivationFunctionType.Tanh
_EXP = mybir.ActivationFunctionType.Exp
_X = mybir.AxisListType.X


def _emit_kernel(ctx: ExitStack, tc: "tile.TileContext", d, stage=4):
    nc = tc.nc

    consts = ctx.enter_context(tc.tile_pool(name="consts", bufs=1))
    setup = ctx.enter_context(tc.tile_pool(name="setup", bufs=1))
    at_pool = ctx.enter_context(tc.tile_pool(name="at_pool", bufs=2))
    an_pool = ctx.enter_context(tc.tile_pool(name="an_pool", bufs=1))
    tanh_pool = ctx.enter_context(tc.tile_pool(name="tanh_pool", bufs=3))
    prod_pool = ctx.enter_context(tc.tile_pool(name="prod_pool", bufs=3))
    e_pool = ctx.enter_context(tc.tile_pool(name="e_pool", bufs=2))
    small = ctx.enter_context(tc.tile_pool(name="small", bufs=2))
    outp = ctx.enter_context(tc.tile_pool(name="outp", bufs=2))
    dram_pool = ctx.enter_context(tc.tile_pool(name="dram_pool", bufs=1, space="DRAM"))
    psum_main = ctx.enter_context(tc.tile_pool(name="psum_main", bufs=2, space="PSUM"))
    psum_s = ctx.enter_context(tc.tile_pool(name="psum_s", bufs=1, space="PSUM"))
    psum_ctx = ctx.enter_context(tc.tile_pool(name="psum_ctx", bufs=1, space="PSUM"))

    # ---- constants (loaded once) ----
    wa_sb = consts.tile([128, NK, U2], BF16, name="wa_sb")
    for kc in range(NK):
        nc.sync.dma_start(out=wa_sb[:, kc, :], in_=d["wa"][kc * 128:(kc + 1) * 128, :])
    vb_sb = consts.tile([128, U2], BF16, name="vb_sb")
    nc.sync.dma_start(out=vb_sb, in_=d["vb"][:, :])
    ones_sb = consts.tile([128, 128], F32, name="ones_sb")
    nc.vector.memset(ones_sb, 1.0)
    msk_sb = consts.tile([128, EX, NT], F32, name="msk_sb")
    for i in range(EX):
        nc.scalar.dma_start(out=msk_sb[:, i, :], in_=d["msk"][i, :, :])

    # ---- dec = h @ Wh_w + Wh_b for all 4 examples ----
    ht_sb = setup.tile([128, NKH, EX], BF16, name="ht_sb")
    whw_sb = setup.tile([128, NKH, U2], BF16, name="whw_sb")
    for kc in range(NKH):
        nc.scalar.dma_start(out=ht_sb[:, kc, :], in_=d["ht"][kc * 128:(kc + 1) * 128, :])
        nc.scalar.dma_start(out=whw_sb[:, kc, :], in_=d["whw"][kc * 128:(kc + 1) * 128, :])
    whb_sb = setup.tile([EX, U2], F32, name="whb_sb")
    nc.scalar.dma_start(out=whb_sb, in_=d["whb"][:, :])

    dec_ps = psum_main.tile([128, U2], F32, name="dec_ps", tag="mm")
    for uh in range(2):
        sl = slice(uh * 512, (uh + 1) * 512)
        for kc in range(NKH):
            nc.tensor.matmul(
                dec_ps[0:EX, sl],
                lhsT=ht_sb[:, kc, :],
                rhs=whw_sb[:, kc, sl],
                start=(kc == 0),
                stop=(kc == NKH - 1),
            )
    dec_sb = setup.tile([EX, U2], F32, name="dec_sb")
    nc.vector.tensor_add(dec_sb, dec_ps[0:EX, :], whb_sb)
    dec_bf = setup.tile([EX, U2], BF16, name="dec_bf")
    nc.vector.tensor_copy(out=dec_bf, in_=dec_sb)

    # ---- augmented-K operands: feats += cov[t]*Wc[u] + 1*dec[u] ----
    # aug_w[k', i, u]: k'=0 -> Wc, k'=1 -> dec_i     (matmul rhs, [2, 512])
    # aug_x[k', i, t]: k'=0 -> coverage_i, k'=1 -> 1 (matmul lhsT, [2, 128])
    # Engine/DMA SBUF APs must start at partition 0, so row 1 (dec) is
    # staged through DRAM and both rows land in one base-0 DMA.
    aug_x = setup.tile([2, EX, TX], BF16, name="aug_x")
    nc.sync.dma_start(out=aug_x, in_=d["covo"][:, :, :])
    augw_d = dram_pool.tile([2, EX, U2], BF16, name="augw_d", tag="augw", bufs=1)
    nc.scalar.dma_start(out=augw_d[0:1, :, :], in_=d["wc_rep"][:, :])
    nc.scalar.dma_start(out=augw_d[1:2, :, :], in_=dec_bf)
    aug_w = setup.tile([2, EX, U2], BF16, name="aug_w")
    nc.scalar.dma_start(out=aug_w, in_=augw_d[:, :, :])

    def emit_tail(i, an_sb, e_cols):
        if stage < 2:
            nc.scalar.dma_start(out=d["out_alpha"][i, :, :], in_=e_cols)
            return
        # softmax over t with masking (max-subtraction skipped; |e| bounded)
        pexp = small.tile([128, NT], F32, name="pexp", tag="pexp")
        nc.scalar.activation(out=pexp, in_=e_cols, func=_EXP)
        pm = small.tile([128, NT], F32, name="pm", tag="pm")
        nc.vector.tensor_mul(pm, pexp, msk_sb[:, i, :])
        pm_sum = small.tile([128, 1], F32, name="pm_sum", tag="pmsum")
        nc.vector.reduce_sum(pm_sum, pm, axis=_X)
        # broadcast S = sum_p pm_sum[p] to all 128 partitions via ones matmul
        s_ps = psum_s.tile([128, 1], F32, name="s_ps", tag="sps")
        nc.tensor.matmul(s_ps, lhsT=ones_sb, rhs=pm_sum, start=True, stop=True)
        r = small.tile([128, 1], F32, name="r", tag="r")
        nc.vector.reciprocal(out=r, in_=s_ps)
        alpha_cols = outp.tile([128, NT], F32, name="alpha_cols", tag="alpha")
        nc.vector.tensor_scalar_mul(alpha_cols, pm, r)
        nc.scalar.dma_start(out=d["out_alpha"][i, :, :], in_=alpha_cols)
        if stage < 3:
            return
        alpha_bf = small.tile([128, NT], BF16, name="alpha_bf", tag="albf")
        nc.vector.tensor_copy(out=alpha_bf, in_=alpha_cols)
        # context = sum_t alpha[t] * a[t, :]
        ctx_ps = psum_ctx.tile([1, U2], F32, name="ctx_ps", tag="ctx")
        for uh in range(2):
            sl = slice(uh * 512, (uh + 1) * 512)
            for j in range(NT):
                nc.tensor.matmul(
                    ctx_ps[:, sl],
                    lhsT=alpha_bf[:, j:j + 1],
                    rhs=an_sb[:, j, sl],
                    start=(j == 0),
                    stop=(j == NT - 1),
                )
        ctx_sb = outp.tile([1, U2], F32, name="ctx_sb", tag="ctxsb")
        nc.scalar.copy(ctx_sb, ctx_ps)
        nc.scalar.dma_start(out=d["out_ctx"][i:i + 1, :], in_=ctx_sb)

    pending = None
    for i in range(EX):
        at_sb = at_pool.tile([128, NK, TX], BF16, name="at_sb", tag="at")
        for kc in range(NK):
            nc.sync.dma_start(out=at_sb[:, kc, :], in_=d["at"][i, kc * 128:(kc + 1) * 128, :])
        an_sb = an_pool.tile([128, NT, U2], BF16, name="an_sb", tag="an")
        for j in range(NT):
            nc.sync.dma_start(out=an_sb[:, j, :], in_=d["an"][i, j * 128:(j + 1) * 128, :])
        e_cols = e_pool.tile([128, NT], F32, name="e_cols", tag="e")

        for j in range(NT):
            jsl = slice(j * 128, (j + 1) * 128)
            ps = psum_main.tile([128, U2], F32, name="ps", tag="mm")
            for uh in range(2):
                sl = slice(uh * 512, (uh + 1) * 512)
                for kc in range(NK):
                    nc.tensor.matmul(
                        ps[:, sl],
                        lhsT=at_sb[:, kc, jsl],
                        rhs=wa_sb[:, kc, sl],
                        start=(kc == 0),
                        stop=False,
                    )
                nc.tensor.matmul(
                    ps[:, sl],
                    lhsT=aug_x[:, i, jsl],
                    rhs=aug_w[:, i, sl],
                    start=False,
                    stop=True,
                )
            th = tanh_pool.tile([128, U2], BF16, name="th", tag="th")
            nc.scalar.activation(out=th, in_=ps, func=_TANH)
            prod = prod_pool.tile([128, U2], BF16, name="prod", tag="prod")
            nc.vector.tensor_mul(prod, th, vb_sb)
            nc.vector.reduce_sum(e_cols[:, j:j + 1], prod, axis=_X)
            if j == 0 and pending is not None:
                # interleave the previous example's tail so PE never stalls
                emit_tail(*pending)
                pending = None
        pending = (i, an_sb, e_cols)
    emit_tail(*pending)


def _build_module(stage=4):
    nc = bacc.Bacc()
    d = {
        "at": nc.dram_tensor("at", [EX, U2, TX], BF16, kind="ExternalInput"),
        "an": nc.dram_tensor("an", [EX, TX, U2], BF16, kind="ExternalInput"),
        "wa": nc.dram_tensor("wa", [U2, U2], BF16, kind="ExternalInput"),
        "whw": nc.dram_tensor("whw", [DH, U2], BF16, kind="ExternalInput"),
        "ht": nc.dram_tensor("ht", [DH, EX], BF16, kind="ExternalInput"),
        "whb": nc.dram_tensor("whb", [EX, U2], F32, kind="ExternalInput"),
        "wc_rep": nc.dram_tensor("wc_rep", [EX, U2], BF16, kind="ExternalInput"),
        "covo": nc.dram_tensor("covo", [2, EX, TX], BF16, kind="ExternalInput"),
        "vb": nc.dram_tensor("vb", [128, U2], BF16, kind="ExternalInput"),
        "msk": nc.dram_tensor("msk", [EX, 128, NT], F32, kind="ExternalInput"),
        "out_alpha": nc.dram_tensor("out_alpha", [EX, 128, NT], F32, kind="ExternalOutput"),
        "out_ctx": nc.dram_tensor("out_ctx", [EX, U2], F32, kind="ExternalOutput"),
    }
    with tile.TileContext(nc) as tc:
        with ExitStack() as ctx:
            _emit_kernel(ctx, tc, d, stage=stage)
    nc.compile()
    return nc


_module_cache = {}


def _get_module(stage=4):
    key = ("nc", stage)
    if key not in _module_cache:
        _module_cache[key] = _build_module(stage=stage)
    return _module_cache[key]


def _prepare_in_maps(a, h, coverage, X_mask, Wa, Wh_w, Wh_b, Wc, v,
                     use_coverage, use_masking):
    a = np.asarray(a, dtype=np.float32)
    h = np.asarray(h, dtype=np.float32)
    coverage = np.asarray(coverage, dtype=np.float32)
    X_mask = np.asarray(X_mask)

    a_bf = a.astype(NP_BF16)
    wa_bf = np.ascontiguousarray(np.asarray(Wa, np.float32).astype(NP_BF16))
    whw_bf = np.ascontiguousarray(np.asarray(Wh_w, np.float32).astype(NP_BF16))
    wc_rep = np.ascontiguousarray(
        np.broadcast_to(np.asarray(Wc, np.float32).reshape(1, U2), (EX, U2))
    ).astype(NP_BF16)
    vb = np.ascontiguousarray(
        np.broadcast_to(np.asarray(v, np.float32).reshape(1, U2), (128, U2))
    ).astype(NP_BF16)
    whb_row = np.asarray(Wh_b, np.float32).reshape(1, U2)

    if int(use_coverage):
        cov_all = coverage.astype(NP_BF16)
    else:
        cov_all = np.zeros((B, TX), dtype=NP_BF16)
    if int(use_masking):
        m_all = X_mask.reshape(B, TX).astype(np.float32)
    else:
        m_all = np.ones((B, TX), dtype=np.float32)
    # cols layout: msk[i, p, j] = m[i, j*128 + p]
    msk_all = np.ascontiguousarray(m_all.reshape(B, NT, 128).transpose(0, 2, 1))

    in_maps = []
    for c in range(NCORES):
        ex = slice(c * EX, (c + 1) * EX)
        in_maps.append({
            "at": np.ascontiguousarray(a_bf[ex].transpose(0, 2, 1)),
            "an": np.ascontiguousarray(a_bf[ex]),
            "wa": wa_bf,
            "whw": whw_bf,
            "ht": np.ascontiguousarray(h[ex].T.astype(NP_BF16)),
            "whb": np.ascontiguousarray(np.broadcast_to(whb_row, (EX, U2))),
            "wc_rep": wc_rep,
            "covo": np.ascontiguousarray(
                np.stack([cov_all[ex], np.ones((EX, TX), NP_BF16)])
            ),
            "vb": vb,
            "msk": msk_all[ex],
        })
    return in_maps


def kernel(a, h, coverage, X_mask, Wa, Wh_w, Wh_b, Wc, v,
           use_coverage, use_masking, _trace=False, _tmpdir=None):
    in_maps = _prepare_in_maps(a, h, coverage, X_mask, Wa, Wh_w, Wh_b, Wc, v,
                               use_coverage, use_masking)
    nc = _get_module()
    res = run_bass_kernel_spmd(
        nc, in_maps, core_ids=list(range(NCORES)),
        trace=_trace, tmpdir=_tmpdir,
    )
    kernel._last_results = res

    context = np.empty((B, 1, U2), dtype=np.float32)
    alpha = np.empty((B, TX), dtype=np.float32)
    for c in range(NCORES):
        out = res.results[c]
        ex = slice(c * EX, (c + 1) * EX)
        context[ex, 0, :] = out["out_ctx"]
        # out_alpha [EX, 128, NT] cols layout (t = j*128 + p) -> [EX, TX]
        alpha[ex] = out["out_alpha"].transpose(0, 2, 1).reshape(EX, TX)
    return context, alpha
